# revision 33
# baseline (speedup 1.0000x reference)
"""Trainium2 Bass kernel for nn_MemoryEfficientAttention (full MHA).

Reference computation (fp32):
    q = split_heads(x @ Wq.T + bq); k, v likewise
    attn = softmax(q @ k.T / sqrt(64))
    out = merge_heads(attn @ v) @ Wo.T + bo

Shapes: B=2, S=4096, D=1024, H=16, head_dim=64.

Sharding across 8 NeuronCores (Megatron attention sharding):
  - 2 heads per core (= 128 of the 1024 projection dims, contiguous slice).
  - Q/K/V projections column-parallel, output projection row-parallel;
    the 8 per-core partial outputs are summed on the host (+ bo).
  - bv never enters the device: softmax rows sum to 1, so its entire effect
    on the output is the constant vector Wo @ bv, added on the host.

Per-core kernel (flash-attention style, nothing S^2-sized touches HBM):
  Phase 1: qT/kT = W_c @ x.T + b_c as fp16 matmuls ([128, S] transposed
           layouts); V projected directly in natural [S, 128] layout
           (x-tile stationary), stored with an interleaved ones-column per
           head (v_aug) that makes the PV matmul accumulate the softmax
           denominator in row 64 for free.
  Phase 2: per (batch, q-chunk of 512): loop over 32 key tiles:
           scoresT[kpos, q] for both heads (row-packed in the PE array,
           they run concurrently), exp on ScalarE (no max subtraction --
           scores are bounded by ~4, the softmax is mathematically
           identical), fp16 PV matmul with M=65. Raw output + denominator
           are copied out of PSUM immediately (fast accumulator release);
           normalization (reciprocal of a DMA-broadcast denominator)
           happens once per batch off the critical path.
  Phase 3: out[s, :] = attnT_c.T @ WoT_c in fp32r (natural layout, clean
           DMA out).
"""

import sys

if "/opt/trn_rl_repo" not in sys.path:
    sys.path.insert(0, "/opt/trn_rl_repo")

import numpy as np

B = 2
S_FULL = 4096
D = 1024
H = 16
HD = 64
NCORES = 8
DC = 128          # head dims per core (2 heads x 64)
SCALE = 1.0 / 8.0  # 1/sqrt(64)


def build_kernel(S=S_FULL):
    """Build the per-core Bass program. Returns the compiled Bacc object."""
    import concourse.bacc as bacc
    import concourse.tile as tile
    from concourse import mybir

    f32 = mybir.dt.float32
    f32r = mybir.dt.float32r
    f16 = mybir.dt.float16
    AF = mybir.ActivationFunctionType

    KT = D // 128       # k-tiles over the projection contraction dim
    SQ = 512            # q-chunk size
    NQC = S // SQ       # q chunks per batch
    NKT = S // 128      # key tiles per batch
    NM = S // 512       # x chunks for projections

    nc = bacc.Bacc("TRN2", target_bir_lowering=False, debug=False,
                   num_devices=NCORES)

    xT = nc.dram_tensor("xT", [B, D, S], f16, kind="ExternalInput").ap()
    wqT = nc.dram_tensor("wqT", [D, DC], f16, kind="ExternalInput").ap()
    wkT = nc.dram_tensor("wkT", [D, DC], f16, kind="ExternalInput").ap()
    wvT = nc.dram_tensor("wvT", [D, DC], f16, kind="ExternalInput").ap()
    woT = nc.dram_tensor("woT", [DC, D], f16, kind="ExternalInput").ap()
    bq = nc.dram_tensor("bq", [DC], f32, kind="ExternalInput").ap()
    bk = nc.dram_tensor("bk", [DC], f32, kind="ExternalInput").ap()
    ones = nc.dram_tensor("ones", [128, 32], f16, kind="ExternalInput").ap()
    part = nc.dram_tensor("part", [B, S, D], f32, kind="ExternalOutput").ap()

    with tile.TileContext(nc) as tc:
        with (
            tc.tile_pool(name="consts", bufs=1) as consts,
            tc.tile_pool(name="xt", bufs=2) as xt_pool,
            tc.tile_pool(name="qkv", bufs=2) as qkv_pool,
            tc.tile_pool(name="exp", bufs=3) as exp_pool,
            tc.tile_pool(name="att", bufs=2) as att_pool,
            tc.tile_pool(name="small", bufs=2) as small_pool,
            tc.tile_pool(name="outs", bufs=3) as out_pool,
            tc.tile_pool(name="bounce", bufs=2, space="DRAM") as dram_pool,
            tc.tile_pool(name="ps_proj", bufs=1, space="PSUM") as ps_proj,
            tc.tile_pool(name="ps_scores", bufs=2, space="PSUM") as ps_scores,
            tc.tile_pool(name="ps_acc", bufs=2, space="PSUM") as ps_acc,
            tc.tile_pool(name="ps_pso", bufs=1, space="PSUM") as ps_pso,
        ):
            # ---- constants ----
            wq_sb = consts.tile([128, KT, DC], f16)
            wk_sb = consts.tile([128, KT, DC], f16)
            wv_sb = consts.tile([128, KT, DC], f16)
            wo_sb = consts.tile([128, D], f16)
            bq_sb = consts.tile([128, 1], f32)
            bk_sb = consts.tile([128, 1], f32)

            # tiny dummy exp so the ACT table set loads during startup DMAs
            warm = consts.tile([128, 1], f32)
            nc.vector.memset(warm[:], 0.0)
            nc.scalar.activation(warm[:], warm[:], AF.Exp, scale=1.0)

            for w_sb, w_dram in ((wq_sb, wqT), (wk_sb, wkT), (wv_sb, wvT)):
                nc.gpsimd.dma_start(
                    out=w_sb[:],
                    in_=w_dram.rearrange("(kt p) m -> p kt m", p=128),
                )
            nc.gpsimd.dma_start(out=wo_sb[:], in_=woT)
            for b_sb, b_dram in ((bq_sb, bq), (bk_sb, bk)):
                nc.gpsimd.dma_start(out=b_sb[:], in_=b_dram.rearrange("(p o) -> p o", o=1))

            state = [None, None]  # per-batch dict of tiles

            def alloc_batch(b):
                qT_sb = qkv_pool.tile([128, S], f16, tag="qT", name=f"qT_{b}")
                kT_sb = qkv_pool.tile([128, S], f16, tag="kT", name=f"kT_{b}")
                # v natural layout, per key-tile: [vA(64) | 1 | vB(64) | 1]
                v_sb = qkv_pool.tile([128, NKT, 130], f16, tag="v",
                                     name=f"v_{b}")
                nc.vector.memset(v_sb[:, :, 64:65], 1.0)
                nc.vector.memset(v_sb[:, :, 129:130], 1.0)
                attT_sb = att_pool.tile([128, S], f16, tag="attT",
                                        name=f"attT_{b}")
                den_dram = dram_pool.tile([2, S], f16, tag="den",
                                          name=f"den_{b}")
                state[b] = dict(qT=qT_sb, kT=kT_sb, v=v_sb, attT=attT_sb,
                                den=den_dram)

            def emit_proj_chunk(b, m):
                """Projections for x columns [m*512, (m+1)*512) of batch b."""
                st = state[b]
                xt = xt_pool.tile([128, KT, 512], f16, tag="xt",
                                  name=f"xt_{b}_{m}")
                xsrc = xT[b][:, m * 512:(m + 1) * 512].rearrange(
                    "(kt p) s -> p kt s", p=128)
                half = KT // 2
                nc.sync.dma_start(out=xt[:, 0:half, :], in_=xsrc[:, 0:half, :])
                nc.gpsimd.dma_start(out=xt[:, half:, :], in_=xsrc[:, half:, :])
                for w_sb, b_sb, dst in (
                    (wq_sb, bq_sb, st["qT"]),
                    (wk_sb, bk_sb, st["kT"]),
                ):
                    ps = ps_proj.tile([128, 512], f32, tag="proj",
                                      name=f"ps_{b}_{m}")
                    for j in range(KT):
                        nc.tensor.matmul(
                            ps[:],
                            lhsT=w_sb[:, j, :],
                            rhs=xt[:, j, :],
                            start=(j == 0),
                            stop=(j == KT - 1),
                        )
                    nc.vector.tensor_scalar_add(
                        dst[:, m * 512:(m + 1) * 512], ps[:], b_sb[:],
                    )
                # V in natural layout: x-tile stationary, Wv moving.
                for t in range(4):
                    psv = ps_proj.tile([128, 512], f32, tag="proj",
                                       name=f"psv_{b}_{m}_{t}")
                    for j in range(KT):
                        nc.tensor.matmul(
                            psv[:, 0:DC],
                            lhsT=xt[:, j, t * 128:(t + 1) * 128],
                            rhs=wv_sb[:, j, :],
                            start=(j == 0),
                            stop=(j == KT - 1),
                        )
                    kt_idx = m * 4 + t
                    nc.vector.tensor_copy(
                        v_sb_ := state[b]["v"][:, kt_idx, 0:64], psv[:, 0:64])
                    nc.vector.tensor_copy(
                        state[b]["v"][:, kt_idx, 65:129], psv[:, 64:128])

            def emit_attn(b, qc):
                """Attention for one q-chunk of 512 rows."""
                st = state[b]
                qT_sb, kT_sb, v_sb = st["qT"], st["kT"], st["v"]
                attT_sb, den_dram = st["attT"], st["den"]
                q0, q1 = qc * SQ, (qc + 1) * SQ
                acc_a = ps_acc.tile([128, SQ], f32, tag="acc",
                                    name=f"acca_{b}_{qc}")
                acc_b = ps_acc.tile([128, SQ], f32, tag="acc",
                                    name=f"accb_{b}_{qc}")
                accs = [acc_a, acc_b]
                for j in range(NKT):
                    k0, k1 = j * 128, (j + 1) * 128
                    pss = ps_scores.tile([128, 2 * SQ], f32, tag="scores",
                                         name=f"pss_{b}_{qc}_{j}")
                    for hh in range(2):
                        nc.tensor.matmul(
                            pss[:, hh * SQ:(hh + 1) * SQ],
                            lhsT=kT_sb[hh * 64:(hh + 1) * 64, k0:k1],
                            rhs=qT_sb[hh * 64:(hh + 1) * 64, q0:q1],
                            start=True, stop=True,
                        )
                    ex = exp_pool.tile([128, 2 * SQ], f16, tag="exp",
                                       name=f"ex_{b}_{qc}_{j}")
                    nc.scalar.activation(ex[:], pss[:], AF.Exp, scale=SCALE)
                    for hh in range(2):
                        nc.tensor.matmul(
                            accs[hh][0:65, :],
                            lhsT=v_sb[:, j, hh * 65:(hh + 1) * 65],
                            rhs=ex[:, hh * SQ:(hh + 1) * SQ],
                            start=(j == 0), stop=(j == NKT - 1),
                        )
                # fast PSUM release: copy raw output + denominator out
                for hh in range(2):
                    nc.vector.tensor_copy(
                        attT_sb[hh * 64:(hh + 1) * 64, q0:q1],
                        accs[hh][0:64, :],
                    )
                    dn = small_pool.tile([1, SQ], f32, tag="dn",
                                         name=f"dn_{b}_{qc}_{hh}")
                    nc.vector.tensor_copy(dn[:], accs[hh][64:65, :])
                    rcp = small_pool.tile([1, SQ], f32, tag="rcp",
                                          name=f"rcp_{b}_{qc}_{hh}")
                    nc.vector.reciprocal_approx_fast(rcp[:], dn[:])
                    rcp16 = small_pool.tile([1, SQ], f16, tag="rcp16",
                                            name=f"rcp16_{b}_{qc}_{hh}")
                    with nc.allow_low_precision(reason="fp16 softmax recip"):
                        nc.vector.tensor_copy(rcp16[:], rcp[:])
                    nc.gpsimd.dma_start(out=den_dram[hh, q0:q1], in_=rcp16[:])

                # broadcast the reciprocal + normalize; resolves during the
                # next chunk's attention
                for hh in range(2):
                    bc = small_pool.tile([128, SQ], f16, tag="bcast",
                                         name=f"bc_{b}_{qc}_{hh}")
                    bch = bc[hh * 64:(hh + 1) * 64, :]
                    rd = den_dram[hh, q0:q1]
                    bcast_src = rd.__class__(
                        tensor=rd.tensor, offset=rd.offset,
                        ap=[[0, 64]] + list(rd.ap),
                    )
                    nc.gpsimd.dma_start(out=bch, in_=bcast_src)
                    nc.vector.tensor_mul(
                        attT_sb[hh * 64:(hh + 1) * 64, q0:q1],
                        attT_sb[hh * 64:(hh + 1) * 64, q0:q1],
                        bch,
                    )

            def emit_tail(b, qc, use_act=False):
                """Output projection for a q-chunk, emitted one chunk late
                so the normalization chain has already resolved and the
                in-order PE never stalls on it."""
                st = state[b]
                attT_sb = st["attT"]
                q0, q1 = qc * SQ, (qc + 1) * SQ
                # output projection for this q-chunk's 4 row-tiles
                for sti in range(SQ // 128):
                    s0 = q0 + sti * 128
                    s1 = s0 + 128
                    for oc in range(D // 512):
                        pso = ps_pso.tile([128, 512], f32, tag="pso",
                                          name=f"pso_{b}_{qc}_{sti}_{oc}")
                        nc.tensor.matmul(
                            pso[:],
                            lhsT=attT_sb[:, s0:s1],
                            rhs=wo_sb[:, oc * 512:(oc + 1) * 512],
                            start=True, stop=True,
                        )
                        ob = out_pool.tile([128, 512], f32, tag="ob",
                                           name=f"ob_{b}_{qc}_{sti}_{oc}")
                        if use_act:
                            nc.scalar.copy(ob[:], pso[:])
                        else:
                            nc.vector.tensor_copy(ob[:], pso[:])
                        nc.sync.dma_start(
                            out=part[b, s0:s1, oc * 512:(oc + 1) * 512],
                            in_=ob[:],
                        )

            # ---- emission schedule: batch 1's projections are interleaved
            # into batch 0's late attention so ScalarE never drains ----
            TAIL_DELAY = 1
            pending = []

            def drain_tails(limit, use_act=False):
                while len(pending) > limit:
                    emit_tail(*pending.pop(0), use_act=use_act)

            alloc_batch(0)
            for m in range(NM):
                emit_proj_chunk(0, m)
            half = NQC // 2
            done_m = 0
            for qc in range(NQC):
                emit_attn(0, qc)
                pending.append((0, qc))
                drain_tails(TAIL_DELAY)
                if NQC >= 4 and qc >= half - 1 and done_m < NM:
                    if state[1] is None:
                        alloc_batch(1)
                    for _ in range(2):
                        if done_m < NM:
                            emit_proj_chunk(1, done_m)
                            done_m += 1
            if state[1] is None:
                alloc_batch(1)
            while done_m < NM:
                emit_proj_chunk(1, done_m)
                done_m += 1
            for qc in range(NQC):
                emit_attn(1, qc)
                pending.append((1, qc))
                drain_tails(TAIL_DELAY)
            drain_tails(0, use_act=True)

    nc.compile()
    return nc


def shard_inputs(x, Wq, bq, Wk, bk, Wv, bv, Wo, bo, S=S_FULL):
    """Host-side sharding: returns list of 8 per-core input dicts."""
    x = np.asarray(x, dtype=np.float32)
    xT = np.ascontiguousarray(x.transpose(0, 2, 1)).astype(np.float16)  # [B, D, S]
    in_maps = []
    for c in range(NCORES):
        sl = slice(c * DC, (c + 1) * DC)
        in_maps.append({
            "xT": xT,
            "wqT": np.ascontiguousarray(np.asarray(Wq)[sl, :].T).astype(np.float16),
            "wkT": np.ascontiguousarray(np.asarray(Wk)[sl, :].T).astype(np.float16),
            "wvT": np.ascontiguousarray(np.asarray(Wv)[sl, :].T).astype(np.float16),
            "woT": np.ascontiguousarray(np.asarray(Wo)[:, sl].T).astype(np.float16),
            "bq": np.ascontiguousarray(np.asarray(bq)[sl], dtype=np.float32),
            "bk": np.ascontiguousarray(np.asarray(bk)[sl], dtype=np.float32),
            "ones": np.ones((128, 32), dtype=np.float16),
        })
    return in_maps


_NC_CACHE = {}


def _get_nc(S=S_FULL):
    if S not in _NC_CACHE:
        _NC_CACHE[S] = build_kernel(S)
    return _NC_CACHE[S]


def kernel(x, Wq, bq, Wk, bk, Wv, bv, Wo, bo, _trace=False, _trace_cores=None):
    from concourse import bass_utils

    nc = _get_nc(S_FULL)
    in_maps = shard_inputs(x, Wq, bq, Wk, bk, Wv, bv, Wo, bo)
    kwargs = {}
    if _trace:
        kwargs = dict(trace=True, trace_cores=_trace_cores or [0])
    res = bass_utils.run_bass_kernel_spmd(
        nc, in_maps, core_ids=list(range(NCORES)), **kwargs)
    out = np.zeros((B, S_FULL, D), dtype=np.float32)
    for c in range(NCORES):
        out += res.results[c]["part"]
    # bv is folded out of the device kernel: softmax rows sum to one, so its
    # contribution to the output is the constant Wo @ bv. Add it with bo here.
    bias = (np.asarray(Wo, dtype=np.float64) @ np.asarray(bv, dtype=np.float64)
            + np.asarray(bo, dtype=np.float64))
    out += bias.astype(np.float32)[None, None, :]
    if _trace:
        kernel._last_results = res
    return out
